# revision 2
# baseline (speedup 1.0000x reference)
"""Message-passing kernel for Trainium2 (8 NeuronCores, data-parallel over batch).

Reference computation (per batch element, C=128 channels, H=128, W=256):
  4 sequential directional scans (down, up, right, left); each scan step is
    out[i] = x[i] + relu(conv1d(out[i-1]))
  with a 'same'-padded K=9 conv1d (C->C) along the non-scan spatial axis.

Design (per core, one batch element):
  - whole image resident in SBUF as [C=128 partitions, H*260] fp32r
    (row stride 260: col 0 = zero, cols [1:257] = data, [257:260] = zero)
  - each scan step: 9 PSUM-accumulated fp32r matmuls (one per conv tap,
    weights stationary per tap, rhs = previous row/carry streamed
    contiguously, >=256-wide streams to stay on the fp32r fast path;
    per-tap psum drain offsets kept 8B-aligned by shifting the rhs base
    through a leading zero column for odd taps)
  - recurrence update x + relu(psum) fused into one DVE
    scalar_tensor_tensor (max(psum,0) + x)
  - filler matmuls into a scratch psum bank keep the PE HAM-warm (2.4 GHz)
    through each step's DVE/semaphore window
  - right/left scans keep a contiguous carry tile; the +x columns are
    prefetched to contiguous tiles by ScalarE ahead of time; left-scan
    output is staged in w-major blocks and streamed to DRAM during the
    scan (host undoes the w-major layout)
"""

import numpy as np

C = 128
H = 128
W = 256
K = 9
RS = 260          # image row stride (fp32 words)
CT = 272          # carry tile width for right/left scans
B = 8
N_CORES = 8
SBLK = 32         # output staging block (columns)

_CACHE = {}


# ---------------------------------------------------------------------------
# workarounds for this walrus build (exit drain / per-instruction wait limits)
# ---------------------------------------------------------------------------

def _patch_tile_drain():
    import concourse.mybir as mybir
    import concourse.tile as tile_mod
    from concourse.vector_clock import ScopedClock

    def _drain_and_barrier(self, tick_clock, wait_clock):
        nc = self.nc
        probe = nc.sync.nop()
        wait_clock.add_sem_waits(
            probe.ins, ScopedClock({None: tick_clock.global_clock})
        )
        si = probe.ins.sync_info
        waits = list(si.on_wait) if si is not None else []
        if si is not None:
            probe.ins.sync_info = mybir.SyncInfo(
                on_wait=[], on_update=list(si.on_update)
            )
        for w in waits:
            wi = nc.sync.nop()
            wi.ins.sync_info = mybir.SyncInfo(on_wait=[w], on_update=[])
        nc.sync.drain()

        nc.all_engine_barrier()
        assert self.sems is not None
        popped = nc._tile_sem_poison_stack.pop()
        assert popped is self._sem_poison
        nc.clear_and_free_semaphores(list(self.sems.allocated().values()))
        nc.all_engine_barrier()

    tile_mod.TileContext._drain_and_barrier = _drain_and_barrier


def _split_waits(nc, max_waits=1):
    """This walrus build allows only one semaphore wait per instruction;
    move excess waits onto nops inserted just before, same engine.  Keep a
    PE-updated semaphore (typically the psum producer, last to arrive) on
    the instruction itself so the chained-nop latency hides behind it."""
    import concourse.mybir as mybir

    ctr = 0
    for f in nc.m.functions:
        for bb in f.blocks:
            insts = bb.instructions
            if not any(
                i.sync_info is not None and len(i.sync_info.on_wait) > max_waits
                for i in insts
            ):
                continue
            new = []
            for inst in insts:
                si = inst.sync_info
                ws = list(si.on_wait) if si is not None else []
                if len(ws) > max_waits:
                    ws.sort(key=lambda w: "PE" in (w.ant_name or ""))
                    extra, keep = ws[:-max_waits], ws[-max_waits:]
                    for j in range(0, len(extra), max_waits):
                        ctr += 1
                        nop = mybir.InstNoOp(
                            name=f"waitsplit-{ctr}",
                            sync_info=mybir.SyncInfo(
                                on_wait=extra[j:j + max_waits], on_update=[]
                            ),
                            bass_nofuse=True,
                            engine=inst.engine,
                        )
                        new.append(nop)
                    inst.sync_info = mybir.SyncInfo(
                        on_wait=keep, on_update=list(si.on_update)
                    )
                new.append(inst)
            bb.instructions = new


# ---------------------------------------------------------------------------
# program construction
# ---------------------------------------------------------------------------

def _build_program(n_fill=3):
    import concourse.bass as bass
    import concourse.mybir as mybir
    from concourse.alu_op_type import AluOpType
    from concourse.tile import TileContext

    _patch_tile_drain()

    f32 = mybir.dt.float32
    f32r = mybir.dt.float32r
    u32 = mybir.dt.uint32

    nc = bass.Bass()
    x_in = nc.declare_dram_parameter("x", [C, H * W], f32r, isOutput=False)
    w_in = {}
    for nm in ("wd", "wu", "wr", "wl"):
        w_in[nm] = nc.declare_dram_parameter(nm, [C, K * C], f32r, isOutput=False)
    # w-major output: y[c, w*H + h]; host transposes back
    y_out = nc.declare_dram_parameter("y", [C, W * H], f32, isOutput=True)

    with TileContext(nc) as tc:
        with (
            tc.tile_pool(name="img", bufs=1) as imgp,
            tc.tile_pool(name="wpool", bufs=1) as wp,
            tc.tile_pool(name="cpool", bufs=1) as cp,
            tc.tile_pool(name="stage", bufs=1) as sp,
            tc.tile_pool(name="psum", bufs=4, space="PSUM") as pp,
            tc.tile_pool(name="fpsum", bufs=2, space="PSUM") as fp,
        ):
            # weights first: the first scan stalls on them, x streams after
            wt = {}
            for nm in ("wd", "wu", "wr", "wl"):
                wt[nm] = wp.tile([C, K * C], f32r, tag=f"wt_{nm}", name=f"wt_{nm}")
                nc.sync.dma_start(out=wt[nm][:], in_=w_in[nm][:])

            img = imgp.tile([C, H * RS], f32r, tag="img")
            img3 = img.rearrange("p (h r) -> p h r", r=RS)
            # zero the per-row guard columns (0 and 257..259)
            nc.vector.memset(img3[:, :, 0:1].bitcast(u32), 0)
            nc.vector.memset(img3[:, :, 257:260].bitcast(u32), 0)
            # load x into the data region, 16-row blocks
            x3 = x_in.rearrange("p (h w) -> p h w", w=W)
            for hb in range(0, H, 16):
                nc.sync.dma_start(
                    out=img3[:, hb:hb + 16, 1:257], in_=x3[:, hb:hb + 16, :]
                )

            # carry tiles for right/left scans: [0]=0, [1:129]=data,
            # [129:137]=0 (conv guard), rest finite junk
            cts = []
            for ci in range(3):
                t = cp.tile([C, CT], f32r, tag=f"ct{ci}", name=f"ct{ci}")
                nc.vector.memset(t[:].bitcast(u32), 0)
                cts.append(t)
            # contiguous prefetched +x columns for right/left scans
            xcols = [
                cp.tile([C, C], f32r, tag=f"xc{ci}", name=f"xc{ci}")
                for ci in range(4)
            ]
            # w-major output staging blocks
            stg = [
                sp.tile([C, SBLK * H], f32, tag=f"stg{ci}", name=f"stg{ci}")
                for ci in range(2)
            ]

            filler_rhs = wt["wd"][:, 0:256]

            def row(i):
                return img3[:, i, :]

            def col(w):
                # image column w: [C, H] stride RS, data offset 1+w
                return img3[:, :, 1 + w]

            def taps(wtile, rhs_even, rhs_odd, ps):
                for t in range(K):
                    s = t - 4
                    wsl = wtile[:, t * C:(t + 1) * C]
                    if s % 2 == 0:
                        nc.tensor.matmul(
                            ps[:, 4 - s:4 - s + rhs_even.shape[-1]],
                            wsl, rhs_even, start=(t == 0), stop=(t == K - 1),
                        )
                    else:
                        nc.tensor.matmul(
                            ps[:, 3 - s:3 - s + rhs_odd.shape[-1]],
                            wsl, rhs_odd, start=(t == 0), stop=(t == K - 1),
                        )

            def fillers(n):
                if not n:
                    return
                fps = fp.tile([C, 256], f32, tag="fps")
                for fi in range(n):
                    nc.tensor.matmul(
                        fps[:], wt["wd"][:, fi * C:(fi + 1) * C], filler_rhs,
                        start=(fi == 0), stop=(fi == n - 1),
                    )

            # ---------------- phase 1 down / phase 2 up --------------------
            for phase, wname, order in (
                (1, "wd", range(1, H)),
                (2, "wu", range(H - 2, -1, -1)),
            ):
                src_off = -1 if phase == 1 else 1
                for i in order:
                    r = row(i + src_off)
                    ps = pp.tile([C, 264], f32, tag="ps")
                    taps(wt[wname], r[:, 1:257], r[:, 0:258], ps)
                    nc.vector.scalar_tensor_tensor(
                        out=row(i)[:, 1:257], in0=ps[:, 4:260], scalar=0.0,
                        in1=row(i)[:, 1:257],
                        op0=AluOpType.max, op1=AluOpType.add,
                    )
                    fillers(n_fill)

            # ---------------- phase 3: right -------------------------------
            carry = cts[0]
            nc.vector.tensor_copy(carry[:, 1:129], col(0))
            # prefetch +x columns (2 ahead)
            nc.scalar.copy(xcols[1 % 4][:], col(1))
            nc.scalar.copy(xcols[2 % 4][:], col(2))
            for w in range(1, W):
                ps = pp.tile([C, 264], f32, tag="ps")
                taps(wt["wr"], carry[:, 1:257], carry[:, 0:258], ps)
                newc = cts[w % 3]
                nc.vector.scalar_tensor_tensor(
                    out=newc[:, 1:129], in0=ps[:, 4:132], scalar=0.0,
                    in1=xcols[w % 4][:], op0=AluOpType.max, op1=AluOpType.add,
                )
                # persist for phase 4's +x reads
                nc.scalar.copy(col(w), newc[:, 1:129])
                if w + 2 < W:
                    nc.scalar.copy(xcols[(w + 2) % 4][:], col(w + 2))
                carry = newc
                fillers(n_fill)

            # ---------------- phase 4: left (stores overlap) ---------------
            def stage_ap(w):
                b = w // SBLK
                return stg[b % 2][:, (w - b * SBLK) * H:(w - b * SBLK + 1) * H]

            def flush(b):
                nc.sync.dma_start(
                    out=y_out[:, b * SBLK * H:(b + 1) * SBLK * H],
                    in_=stg[b % 2][:].bitcast(f32),
                )

            carry = cts[0]
            nc.vector.tensor_copy(carry[:, 1:129], col(W - 1))
            nc.scalar.copy(stage_ap(W - 1), col(W - 1))
            nc.scalar.copy(xcols[(W - 2) % 4][:], col(W - 2))
            nc.scalar.copy(xcols[(W - 3) % 4][:], col(W - 3))
            for w in range(W - 2, -1, -1):
                ps = pp.tile([C, 264], f32, tag="ps")
                taps(wt["wl"], carry[:, 1:257], carry[:, 0:258], ps)
                newc = cts[w % 3]
                nc.vector.scalar_tensor_tensor(
                    out=newc[:, 1:129], in0=ps[:, 4:132], scalar=0.0,
                    in1=xcols[w % 4][:], op0=AluOpType.max, op1=AluOpType.add,
                )
                nc.scalar.copy(stage_ap(w), newc[:, 1:129])
                if w - 2 >= 0:
                    nc.scalar.copy(xcols[(w - 2) % 4][:], col(w - 2))
                carry = newc
                if w % SBLK == 0:
                    flush(w // SBLK)
                fillers(n_fill)

    _split_waits(nc, max_waits=1)
    return nc


def _get_program():
    key = "prog"
    if key not in _CACHE:
        _CACHE[key] = _build_program()
    return _CACHE[key]


# ---------------------------------------------------------------------------
# entry point
# ---------------------------------------------------------------------------

def _prep_w(w):
    # w: (Cout, Cin, K) -> lhsT layout [Cin, K*Cout]
    return np.ascontiguousarray(
        np.transpose(np.asarray(w, np.float32), (1, 2, 0)).reshape(C, K * C)
    )


def _make_in_map(x_img, ws):
    # x_img: (C, H, W); ws: [w_down, w_up, w_right, w_left]
    wd, wu, wr, wl = (_prep_w(w) for w in ws)
    return {
        "x": np.ascontiguousarray(np.asarray(x_img, np.float32).reshape(C, H * W)),
        "wd": wd, "wu": wu, "wr": wr, "wl": wl,
    }


def _postprocess(y_flat):
    # y is w-major [C, W*H]; transpose back to [C, H, W]
    return np.asarray(y_flat, np.float32).reshape(C, W, H).transpose(0, 2, 1)


def kernel(x, w_down, w_up, w_right, w_left, _trace=False):
    from concourse.bass_utils import run_bass_kernel_spmd

    nc = _get_program()

    def prep_w(w):
        # w: (Cout, Cin, K) -> lhsT layout [Cin, K*Cout]
        return np.ascontiguousarray(
            np.transpose(np.asarray(w, np.float32), (1, 2, 0)).reshape(C, K * C)
        )

    wd, wu, wr, wl = (prep_w(w) for w in (w_down, w_up, w_right, w_left))
    x = np.asarray(x, np.float32)
    in_maps = [
        {
            "x": np.ascontiguousarray(x[b].reshape(C, H * W)),
            "wd": wd, "wu": wu, "wr": wr, "wl": wl,
        }
        for b in range(B)
    ]
    res = run_bass_kernel_spmd(
        nc, in_maps, list(range(N_CORES)), trace=_trace
    )
    # y is w-major [C, W*H]; transpose back to [C, H, W]
    out = np.stack(
        [
            res.results[b]["y"].reshape(C, W, H).transpose(0, 2, 1)
            for b in range(B)
        ]
    ).astype(np.float32)
    if _trace:
        return out, res
    return out



# revision 4
# speedup vs baseline: 1.0523x; 1.0523x over previous
"""Message-passing kernel for Trainium2 (8 NeuronCores, data-parallel over batch).

Reference computation (per batch element, C=128 channels, H=128, W=256):
  4 sequential directional scans (down, up, right, left); each scan step is
    out[i] = x[i] + relu(conv1d(out[i-1]))
  with a 'same'-padded K=9 conv1d (C->C) along the non-scan spatial axis.

Design (per core, one batch element), v2:
  - everything fp16 except PSUM (fp32) and drain arithmetic: 16-bit matmuls
    run at full stream rate at ANY width (fp32r needs >=256), enabling
    chunked, software-pipelined steps; fp16 over bf16 for the extra
    mantissa bits (measured rel err ~7e-4 vs 6e-3).
  - image resident in SBUF as [C=128 partitions, 4 + H*260 + 4] fp16
    (per-row: 1 zero guard, 256 data, 3 zero guards; plus 4-col pads at
    both ends) so every row sees 4 zeros on each side.
  - each scan step is split into chunks along the output row; each chunk
    has its own psum tile and 9 fixed-psum / sliding-rhs tap matmuls
    (start/stop per chunk group); chunks overlap by 4 cols (redundant
    compute) so chunk k's drain feeds chunk k's AND k+1's next-step taps.
  - drain = fused max(psum,0)+x on DVE (scalar_tensor_tensor); the next
    step's chunk-k taps wait only on drain-k of the previous step ->
    the DVE+sem+PE-latency window hides behind the other chunks' matmuls.
  - right/left scans: contiguous guarded carry slots [C,136] (pass 3,
    rotating x3) or guarded staging slots (pass 4, [C, SBLK*136] x2,
    DMA-flushed per 32-column block, w-major bf16 out; host transposes
    and upcasts). +x read directly from the image via strided in1.
  - filler matmuls into a scratch psum keep the PE p-state warm through
    each step's drain window.
"""

import numpy as np

C = 128
H = 128
W = 256
K = 9
RS = 260          # image row stride (bf16 words)
IB = 4            # image global base pad
CT = 136          # carry slot width: 4 zeros, 128 data, 4 zeros
B = 8
N_CORES = 8
SBLK = 32         # output staging block (columns)

# chunk boundaries (even) for down/up (row width 256) and right/left (128),
# tuned on HW (probe3-5): du 3 chunks = 1131 ns/step, rl 2 chunks = 964.
M_DU = (0, 88, 176, 256)
M_RL = (0, 64, 128)
NF_DU = 0         # fillers per down/up step (PE-bound; none needed)
FW_DU = 128
NF_RL = 1         # fillers per right/left step (window-bound; p-state insurance)
FW_RL = 128

_CACHE = {}


# ---------------------------------------------------------------------------
# workarounds for this walrus build (exit drain / per-instruction wait limits)
# ---------------------------------------------------------------------------

def _patch_tile_drain():
    import concourse.mybir as mybir
    import concourse.tile as tile_mod
    from concourse.vector_clock import ScopedClock

    def _drain_and_barrier(self, tick_clock, wait_clock):
        nc = self.nc
        probe = nc.sync.nop()
        wait_clock.add_sem_waits(
            probe.ins, ScopedClock({None: tick_clock.global_clock})
        )
        si = probe.ins.sync_info
        waits = list(si.on_wait) if si is not None else []
        if si is not None:
            probe.ins.sync_info = mybir.SyncInfo(
                on_wait=[], on_update=list(si.on_update)
            )
        for w in waits:
            wi = nc.sync.nop()
            wi.ins.sync_info = mybir.SyncInfo(on_wait=[w], on_update=[])
        nc.sync.drain()

        nc.all_engine_barrier()
        assert self.sems is not None
        popped = nc._tile_sem_poison_stack.pop()
        assert popped is self._sem_poison
        nc.clear_and_free_semaphores(list(self.sems.allocated().values()))
        nc.all_engine_barrier()

    tile_mod.TileContext._drain_and_barrier = _drain_and_barrier


def _split_waits(nc, max_waits=1):
    """This walrus build allows only one semaphore wait per instruction;
    move excess waits onto nops inserted just before, same engine.  Keep a
    PE-updated semaphore (typically last to arrive) on the instruction
    itself so the chained-nop latency hides behind it."""
    import concourse.mybir as mybir

    ctr = 0
    for f in nc.m.functions:
        for bb in f.blocks:
            insts = bb.instructions
            if not any(
                i.sync_info is not None and len(i.sync_info.on_wait) > max_waits
                for i in insts
            ):
                continue
            new = []
            for inst in insts:
                si = inst.sync_info
                ws = list(si.on_wait) if si is not None else []
                if len(ws) > max_waits:
                    ws.sort(key=lambda w: "PE" in (w.ant_name or ""))
                    extra, keep = ws[:-max_waits], ws[-max_waits:]
                    for j in range(0, len(extra), max_waits):
                        ctr += 1
                        nop = mybir.InstNoOp(
                            name=f"waitsplit-{ctr}",
                            sync_info=mybir.SyncInfo(
                                on_wait=extra[j:j + max_waits], on_update=[]
                            ),
                            bass_nofuse=True,
                            engine=inst.engine,
                        )
                        new.append(nop)
                    inst.sync_info = mybir.SyncInfo(
                        on_wait=keep, on_update=list(si.on_update)
                    )
                new.append(inst)
            bb.instructions = new


# ---------------------------------------------------------------------------
# program construction
# ---------------------------------------------------------------------------

def _build_program():
    import concourse.bass as bass
    import concourse.mybir as mybir
    from concourse.alu_op_type import AluOpType
    from concourse.tile import TileContext

    _patch_tile_drain()

    f32 = mybir.dt.float32
    bf = mybir.dt.float16
    u32 = mybir.dt.uint32

    nc = bass.Bass()
    x_in = nc.declare_dram_parameter("x", [C, H * W], bf, isOutput=False)
    w_in = {}
    for nm in ("wd", "wu", "wr", "wl"):
        w_in[nm] = nc.declare_dram_parameter(nm, [C, K * C], bf, isOutput=False)
    # w-major output: y[c, w*H + h] bf16; host transposes + upcasts
    y_out = nc.declare_dram_parameter("y", [C, W * H], bf, isOutput=True)

    IMGW = IB + H * RS + 4

    with TileContext(nc) as tc:
        with (
            tc.tile_pool(name="img", bufs=1) as imgp,
            tc.tile_pool(name="wpool", bufs=1) as wp,
            tc.tile_pool(name="cpool", bufs=1) as cp,
            tc.tile_pool(name="stage", bufs=1) as sp,
            tc.tile_pool(name="psum", bufs=1, space="PSUM") as pp,
            tc.tile_pool(name="fpsum", bufs=1, space="PSUM") as fp,
        ):
            # weights first: the first scan stalls on them, x streams after
            wt = {}
            for nm in ("wd", "wu", "wr", "wl"):
                wt[nm] = wp.tile([C, K * C], bf, tag=f"wt_{nm}", name=f"wt_{nm}")
                nc.sync.dma_start(out=wt[nm][:], in_=w_in[nm][:])

            img = imgp.tile([C, IMGW], bf, tag="img")
            img3 = img[:, IB:IB + H * RS].rearrange("p (h r) -> p h r", r=RS)
            # zero pads + per-row guard columns
            nc.vector.memset(img[:, 0:IB].bitcast(u32), 0)
            nc.vector.memset(img[:, IB + H * RS:].bitcast(u32), 0)
            nc.vector.memset(img3[:, :, 0:1], 0)
            nc.vector.memset(img3[:, :, 257:260], 0)
            # load x into the data region, 16-row blocks
            x3 = x_in.rearrange("p (h w) -> p h w", w=W)
            for hb in range(0, H, 16):
                nc.sync.dma_start(
                    out=img3[:, hb:hb + 16, 1:257], in_=x3[:, hb:hb + 16, :]
                )

            # carry slots for the right scan
            cts = []
            for ci in range(3):
                t = cp.tile([C, CT], bf, tag=f"ct{ci}", name=f"ct{ci}")
                nc.vector.memset(t[:].bitcast(u32), 0)
                cts.append(t)
            # w-major guarded staging slots for the left scan
            stg = []
            for ci in range(2):
                t = sp.tile([C, SBLK * CT], bf, tag=f"stg{ci}", name=f"stg{ci}")
                t3 = t.rearrange("p (s r) -> p s r", r=CT)
                nc.vector.memset(t3[:, :, 0:4].bitcast(u32), 0)
                nc.vector.memset(t3[:, :, 132:136].bitcast(u32), 0)
                stg.append(t)

            # psum tiles: one per chunk, single-buffered (WAR == RAW dep)
            def mk_ps(tag, m):
                ts = []
                for k in range(len(m) - 1):
                    wk = (m[k + 1] + 4 if k < len(m) - 2 else m[-1]) - m[k]
                    ts.append(pp.tile([C, wk], f32, tag=f"{tag}{k}",
                                      name=f"{tag}{k}"))
                return ts

            pd = mk_ps("pd", M_DU)
            pr = mk_ps("pr", M_RL)
            fps = fp.tile([C, 256], f32, tag="fps", name="fps")

            filler_rhs = wt["wd"][:, 0:256]

            def fillers(n, fw):
                for fi in range(n):
                    nc.tensor.matmul(
                        fps[:, 0:fw], wt["wd"][:, fi * C:(fi + 1) * C],
                        filler_rhs[:, 0:fw], start=True, stop=True,
                    )

            def row_base(h):
                # img col index of row h's data col 0
                return IB + h * RS + 1

            def chunk_taps(wtile, m, ps_tiles, rhs_base_fn):
                """Emit per-chunk tap matmuls. rhs_base_fn(off) -> AP for
                [C, width] rhs starting at data col `off` (may be negative:
                guards)."""
                nch = len(m) - 1
                for k in range(nch):
                    a = m[k]
                    bw = (m[k + 1] + 4 if k < nch - 1 else m[-1]) - a
                    for t in range(K):
                        s = t - 4
                        nc.tensor.matmul(
                            ps_tiles[k][:, 0:bw],
                            wtile[:, t * C:(t + 1) * C],
                            rhs_base_fn(a + s, bw),
                            start=(t == 0), stop=(t == K - 1),
                        )

            def chunk_drains(m, ps_tiles, out_fn, x_fn):
                """Per-chunk fused drains: out = max(psum,0) + x.
                out_fn/x_fn(lo, hi) -> AP covering out cols [lo, hi)."""
                nch = len(m) - 1
                for k in range(nch):
                    lo = m[k] + (4 if k > 0 else 0)
                    hi = m[k + 1] + (4 if k < nch - 1 else 0)
                    plo = lo - m[k]
                    nc.vector.scalar_tensor_tensor(
                        out=out_fn(lo, hi),
                        in0=ps_tiles[k][:, plo:plo + hi - lo],
                        scalar=0.0,
                        in1=x_fn(lo, hi),
                        op0=AluOpType.max, op1=AluOpType.add,
                    )

            # ---------------- phase 1 down / phase 2 up --------------------
            for phase, wname, order in (
                (1, "wd", range(1, H)),
                (2, "wu", range(H - 2, -1, -1)),
            ):
                src_off = -1 if phase == 1 else 1
                for i in order:
                    sb = row_base(i + src_off)
                    db = row_base(i)

                    chunk_taps(
                        wt[wname], M_DU, pd,
                        lambda off, bw: img[:, sb + off: sb + off + bw],
                    )
                    chunk_drains(
                        M_DU, pd,
                        lambda lo, hi: img[:, db + lo: db + hi],
                        lambda lo, hi: img[:, db + lo: db + hi],
                    )
                    fillers(NF_DU, FW_DU)

            # ---------------- phase 3: right -------------------------------
            def img_col(w, lo, hi):
                # [C, hi-lo] strided view of image column w, rows [lo, hi)
                return img3[:, lo:hi, 1 + w]

            nc.scalar.copy(cts[0][:, 4:132], img_col(0, 0, H))
            for w in range(1, W):
                prev, new = cts[(w - 1) % 3], cts[w % 3]
                chunk_taps(
                    wt["wr"], M_RL, pr,
                    lambda off, bw: prev[:, 4 + off: 4 + off + bw],
                )
                chunk_drains(
                    M_RL, pr,
                    lambda lo, hi: new[:, 4 + lo: 4 + hi],
                    lambda lo, hi: img_col(w, lo, hi),
                )
                # persist for phase 4's +x reads
                nc.scalar.copy(img_col(w, 0, H), new[:, 4:132])
                fillers(NF_RL, FW_RL)

            # ---------------- phase 4: left (stores overlap) ---------------
            def slot(w):
                b = w // SBLK
                return stg[b % 2][:, (w - b * SBLK) * CT:(w - b * SBLK + 1) * CT]

            def flush(b):
                t3 = stg[b % 2].rearrange("p (s r) -> p s r", r=CT)
                nc.sync.dma_start(
                    out=y_out[:, b * SBLK * H:(b + 1) * SBLK * H],
                    in_=t3[:, :, 4:132],
                )

            nc.scalar.copy(slot(W - 1)[:, 4:132], img_col(W - 1, 0, H))
            for w in range(W - 2, -1, -1):
                prev, new = slot(w + 1), slot(w)
                chunk_taps(
                    wt["wl"], M_RL, pr,
                    lambda off, bw: prev[:, 4 + off: 4 + off + bw],
                )
                chunk_drains(
                    M_RL, pr,
                    lambda lo, hi: new[:, 4 + lo: 4 + hi],
                    lambda lo, hi: img_col(w, lo, hi),
                )
                if w % SBLK == 0:
                    flush(w // SBLK)
                fillers(NF_RL, FW_RL)

    _split_waits(nc, max_waits=1)
    return nc


def _get_program():
    key = "prog"
    if key not in _CACHE:
        _CACHE[key] = _build_program()
    return _CACHE[key]


# ---------------------------------------------------------------------------
# entry point
# ---------------------------------------------------------------------------

def _prep_w(w):
    # w: (Cout, Cin, K) -> lhsT layout [Cin, K*Cout], bf16
    return np.ascontiguousarray(
        np.transpose(np.asarray(w, np.float32), (1, 2, 0)).reshape(C, K * C)
    ).astype(np.float16)


def _make_in_map(x_img, ws):
    # x_img: (C, H, W); ws: [w_down, w_up, w_right, w_left]
    wd, wu, wr, wl = (_prep_w(w) for w in ws)
    return {
        "x": np.ascontiguousarray(
            np.asarray(x_img, np.float32).reshape(C, H * W)
        ).astype(np.float16),
        "wd": wd, "wu": wu, "wr": wr, "wl": wl,
    }


def _postprocess(y_flat):
    # y is w-major bf16 [C, W*H]; transpose back to [C, H, W] fp32
    return (
        np.asarray(y_flat).astype(np.float32).reshape(C, W, H).transpose(0, 2, 1)
    )


def kernel(x, w_down, w_up, w_right, w_left, _trace=False):
    from concourse.bass_utils import run_bass_kernel_spmd

    nc = _get_program()

    x = np.asarray(x, np.float32)
    ws = [w_down, w_up, w_right, w_left]
    in_maps = [_make_in_map(x[b], ws) for b in range(B)]
    res = run_bass_kernel_spmd(nc, in_maps, list(range(N_CORES)), trace=_trace)
    out = np.stack([_postprocess(res.results[b]["y"]) for b in range(B)])
    if _trace:
        return out, res
    return out


# revision 7
# speedup vs baseline: 1.0590x; 1.0064x over previous
"""Message-passing kernel for Trainium2 (8 NeuronCores, data-parallel over batch).

Reference computation (per batch element, C=128 channels, H=128, W=256):
  4 sequential directional scans (down, up, right, left); each scan step is
    out[i] = x[i] + relu(conv1d(out[i-1]))
  with a 'same'-padded K=9 conv1d (C->C) along the non-scan spatial axis.

Design (per core, one batch element), v2:
  - everything fp16 except PSUM (fp32) and drain arithmetic: 16-bit matmuls
    run at full stream rate at ANY width (fp32r needs >=256), enabling
    chunked, software-pipelined steps; fp16 over bf16 for the extra
    mantissa bits (measured rel err ~7e-4 vs 6e-3).
  - image resident in SBUF as [C=128 partitions, 4 + H*260 + 4] fp16
    (per-row: 1 zero guard, 256 data, 3 zero guards; plus 4-col pads at
    both ends) so every row sees 4 zeros on each side.
  - each scan step is split into chunks along the output row; each chunk
    has its own psum tile and 9 fixed-psum / sliding-rhs tap matmuls
    (start/stop per chunk group); chunks overlap by 4 cols (redundant
    compute) so chunk k's drain feeds chunk k's AND k+1's next-step taps.
  - drain = fused max(psum,0)+x on DVE (scalar_tensor_tensor); the next
    step's chunk-k taps wait only on drain-k of the previous step ->
    the DVE+sem+PE-latency window hides behind the other chunks' matmuls.
  - right/left scans: contiguous guarded carry slots [C,136] (pass 3,
    rotating x3) or guarded staging slots (pass 4, [C, SBLK*136] x2,
    DMA-flushed per 32-column block, w-major fp16 out; host transposes
    and upcasts). +x read directly from the image via strided in1.
  - filler matmuls into a scratch psum keep the PE p-state warm through
    each step's drain window.
"""

import numpy as np

C = 128
H = 128
W = 256
K = 9
RS = 260          # image row stride (fp16 words)
IB = 4            # image global base pad
CT = 136          # carry slot width: 4 zeros, 128 data, 4 zeros
B = 8
N_CORES = 8
SBLK = 32         # output staging block (columns)

# chunk boundaries (even) for down/up (row width 256) and right/left (128),
# tuned on HW (probe3-5): du 3 chunks = 1131 ns/step, rl 2 chunks = 964.
M_DU = (0, 88, 176, 256)
M_RL = (0, 64, 128)
NF_DU = 0         # fillers per down/up step (PE-bound; none needed)
FW_DU = 128
NF_RL = 1         # fillers per right/left step (window-bound; p-state insurance)
FW_RL = 128

_CACHE = {}


# ---------------------------------------------------------------------------
# workarounds for this walrus build (exit drain / per-instruction wait limits)
# ---------------------------------------------------------------------------

def _patch_tile_drain():
    import concourse.mybir as mybir
    import concourse.tile as tile_mod
    from concourse.vector_clock import ScopedClock

    def _drain_and_barrier(self, tick_clock, wait_clock):
        nc = self.nc
        probe = nc.sync.nop()
        wait_clock.add_sem_waits(
            probe.ins, ScopedClock({None: tick_clock.global_clock})
        )
        si = probe.ins.sync_info
        waits = list(si.on_wait) if si is not None else []
        if si is not None:
            probe.ins.sync_info = mybir.SyncInfo(
                on_wait=[], on_update=list(si.on_update)
            )
        for w in waits:
            wi = nc.sync.nop()
            wi.ins.sync_info = mybir.SyncInfo(on_wait=[w], on_update=[])
        nc.sync.drain()

        nc.all_engine_barrier()
        assert self.sems is not None
        popped = nc._tile_sem_poison_stack.pop()
        assert popped is self._sem_poison
        nc.clear_and_free_semaphores(list(self.sems.allocated().values()))
        nc.all_engine_barrier()

    tile_mod.TileContext._drain_and_barrier = _drain_and_barrier


def _split_waits(nc, max_waits=1):
    """This walrus build allows only one semaphore wait per instruction;
    move excess waits onto nops inserted just before, same engine.  Keep a
    PE-updated semaphore (typically last to arrive) on the instruction
    itself so the chained-nop latency hides behind it."""
    import concourse.mybir as mybir

    ctr = 0
    for f in nc.m.functions:
        for bb in f.blocks:
            insts = bb.instructions
            if not any(
                i.sync_info is not None and len(i.sync_info.on_wait) > max_waits
                for i in insts
            ):
                continue
            new = []
            for inst in insts:
                si = inst.sync_info
                ws = list(si.on_wait) if si is not None else []
                if len(ws) > max_waits:
                    ws.sort(key=lambda w: "PE" in (w.ant_name or ""))
                    extra, keep = ws[:-max_waits], ws[-max_waits:]
                    for j in range(0, len(extra), max_waits):
                        ctr += 1
                        nop = mybir.InstNoOp(
                            name=f"waitsplit-{ctr}",
                            sync_info=mybir.SyncInfo(
                                on_wait=extra[j:j + max_waits], on_update=[]
                            ),
                            bass_nofuse=True,
                            engine=inst.engine,
                        )
                        new.append(nop)
                    inst.sync_info = mybir.SyncInfo(
                        on_wait=keep, on_update=list(si.on_update)
                    )
                new.append(inst)
            bb.instructions = new


# ---------------------------------------------------------------------------
# program construction
# ---------------------------------------------------------------------------

def _build_program():
    import concourse.bass as bass
    import concourse.mybir as mybir
    from concourse.alu_op_type import AluOpType
    from concourse.tile import TileContext

    _patch_tile_drain()

    f32 = mybir.dt.float32
    bf = mybir.dt.float16
    u32 = mybir.dt.uint32

    nc = bass.Bass()
    x_in = nc.declare_dram_parameter("x", [C, H * W], bf, isOutput=False)
    w_in = {}
    for nm in ("wd", "wu", "wr", "wl"):
        w_in[nm] = nc.declare_dram_parameter(nm, [C, K * C], bf, isOutput=False)
    # w-major output: y[c, w*H + h] fp16; host transposes + upcasts
    y_out = nc.declare_dram_parameter("y", [C, W * H], bf, isOutput=True)

    IMGW = IB + H * RS + 4

    with TileContext(nc) as tc:
        with (
            tc.tile_pool(name="img", bufs=1) as imgp,
            tc.tile_pool(name="wpool", bufs=1) as wp,
            tc.tile_pool(name="cpool", bufs=1) as cp,
            tc.tile_pool(name="stage", bufs=1) as sp,
            tc.tile_pool(name="psum", bufs=1, space="PSUM") as pp,
            tc.tile_pool(name="fpsum", bufs=1, space="PSUM") as fp,
        ):
            # wd first: the first scan stalls on it; the other weights and
            # x stream behind it
            wt = {}
            for nm in ("wd", "wu", "wr", "wl"):
                wt[nm] = wp.tile([C, K * C], bf, tag=f"wt_{nm}", name=f"wt_{nm}")
            nc.sync.dma_start(out=wt["wd"][:], in_=w_in["wd"][:])

            img = imgp.tile([C, IMGW], bf, tag="img")
            img3 = img[:, IB:IB + H * RS].rearrange("p (h r) -> p h r", r=RS)
            # zero pads + per-row guard columns
            nc.vector.memset(img[:, 0:IB].bitcast(u32), 0)
            nc.vector.memset(img[:, IB + H * RS:].bitcast(u32), 0)
            nc.vector.memset(img3[:, :, 0:1], 0)
            nc.vector.memset(img3[:, :, 257:260], 0)
            # load x into the data region, 16-row blocks
            x3 = x_in.rearrange("p (h w) -> p h w", w=W)
            for hb in range(0, H, 16):
                nc.sync.dma_start(
                    out=img3[:, hb:hb + 16, 1:257], in_=x3[:, hb:hb + 16, :]
                )
                if hb == 0:
                    for nm in ("wu", "wr", "wl"):
                        nc.sync.dma_start(out=wt[nm][:], in_=w_in[nm][:])

            # carry slots for the right scan
            cts = []
            for ci in range(3):
                t = cp.tile([C, CT], bf, tag=f"ct{ci}", name=f"ct{ci}")
                nc.vector.memset(t[:].bitcast(u32), 0)
                cts.append(t)
            # w-major guarded staging slots for the left scan
            stg = []
            for ci in range(2):
                t = sp.tile([C, SBLK * CT], bf, tag=f"stg{ci}", name=f"stg{ci}")
                t3 = t.rearrange("p (s r) -> p s r", r=CT)
                nc.vector.memset(t3[:, :, 0:4].bitcast(u32), 0)
                nc.vector.memset(t3[:, :, 132:136].bitcast(u32), 0)
                stg.append(t)

            # psum tiles: one per chunk, single-buffered (WAR == RAW dep)
            def mk_ps(tag, m):
                ts = []
                for k in range(len(m) - 1):
                    wk = (m[k + 1] + 4 if k < len(m) - 2 else m[-1]) - m[k]
                    ts.append(pp.tile([C, wk], f32, tag=f"{tag}{k}",
                                      name=f"{tag}{k}"))
                return ts

            pd = mk_ps("pd", M_DU)
            pr = mk_ps("pr", M_RL)
            fps = fp.tile([C, 256], f32, tag="fps", name="fps")

            filler_rhs = wt["wd"][:, 0:256]

            def fillers(n, fw):
                for fi in range(n):
                    nc.tensor.matmul(
                        fps[:, 0:fw], wt["wd"][:, fi * C:(fi + 1) * C],
                        filler_rhs[:, 0:fw], start=True, stop=True,
                    )

            def row_base(h):
                # img col index of row h's data col 0
                return IB + h * RS + 1

            def chunk_taps(wtile, m, ps_tiles, rhs_base_fn):
                """Emit per-chunk tap matmuls. rhs_base_fn(off) -> AP for
                [C, width] rhs starting at data col `off` (may be negative:
                guards)."""
                nch = len(m) - 1
                for k in range(nch):
                    a = m[k]
                    bw = (m[k + 1] + 4 if k < nch - 1 else m[-1]) - a
                    for t in range(K):
                        s = t - 4
                        nc.tensor.matmul(
                            ps_tiles[k][:, 0:bw],
                            wtile[:, t * C:(t + 1) * C],
                            rhs_base_fn(a + s, bw),
                            start=(t == 0), stop=(t == K - 1),
                        )

            def chunk_drains(m, ps_tiles, out_fn, x_fn):
                """Per-chunk fused drains: out = max(psum,0) + x.
                out_fn/x_fn(lo, hi) -> AP covering out cols [lo, hi)."""
                nch = len(m) - 1
                for k in range(nch):
                    lo = m[k] + (4 if k > 0 else 0)
                    hi = m[k + 1] + (4 if k < nch - 1 else 0)
                    plo = lo - m[k]
                    nc.vector.scalar_tensor_tensor(
                        out=out_fn(lo, hi),
                        in0=ps_tiles[k][:, plo:plo + hi - lo],
                        scalar=0.0,
                        in1=x_fn(lo, hi),
                        op0=AluOpType.max, op1=AluOpType.add,
                    )

            # ---------------- phase 1 down / phase 2 up --------------------
            for phase, wname, order in (
                (1, "wd", range(1, H)),
                (2, "wu", range(H - 2, -1, -1)),
            ):
                src_off = -1 if phase == 1 else 1
                for i in order:
                    sb = row_base(i + src_off)
                    db = row_base(i)

                    chunk_taps(
                        wt[wname], M_DU, pd,
                        lambda off, bw: img[:, sb + off: sb + off + bw],
                    )
                    chunk_drains(
                        M_DU, pd,
                        lambda lo, hi: img[:, db + lo: db + hi],
                        lambda lo, hi: img[:, db + lo: db + hi],
                    )
                    fillers(NF_DU, FW_DU)

            # ---------------- phase 3: right -------------------------------
            def img_col(w, lo, hi):
                # [C, hi-lo] strided view of image column w, rows [lo, hi)
                return img3[:, lo:hi, 1 + w]

            nc.scalar.copy(cts[0][:, 4:132], img_col(0, 0, H))
            for w in range(1, W):
                prev, new = cts[(w - 1) % 3], cts[w % 3]
                chunk_taps(
                    wt["wr"], M_RL, pr,
                    lambda off, bw: prev[:, 4 + off: 4 + off + bw],
                )
                chunk_drains(
                    M_RL, pr,
                    lambda lo, hi: new[:, 4 + lo: 4 + hi],
                    lambda lo, hi: img_col(w, lo, hi),
                )
                # persist for phase 4's +x reads
                nc.scalar.copy(img_col(w, 0, H), new[:, 4:132])
                fillers(NF_RL, FW_RL)

            # ---------------- phase 4: left (stores overlap) ---------------
            def slot(w):
                b = w // SBLK
                return stg[b % 2][:, (w - b * SBLK) * CT:(w - b * SBLK + 1) * CT]

            def flush(b):
                t3 = stg[b % 2].rearrange("p (s r) -> p s r", r=CT)
                nc.sync.dma_start(
                    out=y_out[:, b * SBLK * H:(b + 1) * SBLK * H],
                    in_=t3[:, :, 4:132],
                )

            nc.scalar.copy(slot(W - 1)[:, 4:132], img_col(W - 1, 0, H))
            for w in range(W - 2, -1, -1):
                prev, new = slot(w + 1), slot(w)
                chunk_taps(
                    wt["wl"], M_RL, pr,
                    lambda off, bw: prev[:, 4 + off: 4 + off + bw],
                )
                chunk_drains(
                    M_RL, pr,
                    lambda lo, hi: new[:, 4 + lo: 4 + hi],
                    lambda lo, hi: img_col(w, lo, hi),
                )
                if w % SBLK == 0:
                    flush(w // SBLK)
                fillers(NF_RL, FW_RL)

    _split_waits(nc, max_waits=1)
    return nc


def _get_program():
    key = "prog"
    if key not in _CACHE:
        _CACHE[key] = _build_program()
    return _CACHE[key]


# ---------------------------------------------------------------------------
# entry point
# ---------------------------------------------------------------------------

def _prep_w(w):
    # w: (Cout, Cin, K) -> lhsT layout [Cin, K*Cout], fp16
    return np.ascontiguousarray(
        np.transpose(np.asarray(w, np.float32), (1, 2, 0)).reshape(C, K * C)
    ).astype(np.float16)


def _make_in_map(x_img, ws):
    # x_img: (C, H, W); ws: [w_down, w_up, w_right, w_left]
    wd, wu, wr, wl = (_prep_w(w) for w in ws)
    return {
        "x": np.ascontiguousarray(
            np.asarray(x_img, np.float32).reshape(C, H * W)
        ).astype(np.float16),
        "wd": wd, "wu": wu, "wr": wr, "wl": wl,
    }


def _postprocess(y_flat):
    # y is w-major fp16 [C, W*H]; transpose back to [C, H, W] fp32
    return (
        np.asarray(y_flat).astype(np.float32).reshape(C, W, H).transpose(0, 2, 1)
    )


def kernel(x, w_down, w_up, w_right, w_left, _trace=False):
    from concourse.bass_utils import run_bass_kernel_spmd

    nc = _get_program()

    x = np.asarray(x, np.float32)
    ws = [w_down, w_up, w_right, w_left]
    in_maps = [_make_in_map(x[b], ws) for b in range(B)]
    res = run_bass_kernel_spmd(nc, in_maps, list(range(N_CORES)), trace=_trace)
    out = np.stack([_postprocess(res.results[b]["y"]) for b in range(B)])
    if _trace:
        return out, res
    return out


# revision 8
# speedup vs baseline: 1.0658x; 1.0064x over previous
"""Message-passing kernel for Trainium2 (8 NeuronCores, data-parallel over batch).

Reference computation (per batch element, C=128 channels, H=128, W=256):
  4 sequential directional scans (down, up, right, left); each scan step is
    out[i] = x[i] + relu(conv1d(out[i-1]))
  with a 'same'-padded K=9 conv1d (C->C) along the non-scan spatial axis.

Design (per core, one batch element), v2:
  - everything fp16 except PSUM (fp32) and drain arithmetic: 16-bit matmuls
    run at full stream rate at ANY width (fp32r needs >=256), enabling
    chunked, software-pipelined steps; fp16 over bf16 for the extra
    mantissa bits (measured rel err ~7e-4 vs 6e-3).
  - image resident in SBUF as [C=128 partitions, 4 + H*260 + 4] fp16
    (per-row: 1 zero guard, 256 data, 3 zero guards; plus 4-col pads at
    both ends) so every row sees 4 zeros on each side.
  - each scan step is split into chunks along the output row; each chunk
    has its own psum tile and 9 fixed-psum / sliding-rhs tap matmuls
    (start/stop per chunk group); chunks overlap by 4 cols (redundant
    compute) so chunk k's drain feeds chunk k's AND k+1's next-step taps.
  - drain = fused max(psum,0)+x on DVE (scalar_tensor_tensor); the next
    step's chunk-k taps wait only on drain-k of the previous step ->
    the DVE+sem+PE-latency window hides behind the other chunks' matmuls.
  - right/left scans: contiguous guarded carry slots [C,136] (pass 3,
    rotating x3) or guarded staging slots (pass 4, [C, SBLK*136] x2,
    DMA-flushed per 32-column block, w-major fp16 out; host transposes
    and upcasts). +x read directly from the image via strided in1.
  - filler matmuls into a scratch psum keep the PE p-state warm through
    each step's drain window.
"""

import numpy as np

C = 128
H = 128
W = 256
K = 9
RS = 260          # image row stride (fp16 words)
IB = 4            # image global base pad
CT = 136          # carry slot width: 4 zeros, 128 data, 4 zeros
B = 8
N_CORES = 8
SBLK = 16         # output staging block (columns)

# chunk boundaries (even) for down/up (row width 256) and right/left (128),
# tuned on HW (probe3-5): du 3 chunks = 1131 ns/step, rl 2 chunks = 964.
M_DU = (0, 88, 176, 256)
M_RL = (0, 64, 128)
NF_DU = 0         # fillers per down/up step (PE-bound; none needed)
FW_DU = 128
NF_RL = 1         # fillers per right/left step (window-bound; p-state insurance)
FW_RL = 128

_CACHE = {}


# ---------------------------------------------------------------------------
# workarounds for this walrus build (exit drain / per-instruction wait limits)
# ---------------------------------------------------------------------------

def _patch_tile_drain():
    import concourse.mybir as mybir
    import concourse.tile as tile_mod
    from concourse.vector_clock import ScopedClock

    def _drain_and_barrier(self, tick_clock, wait_clock):
        nc = self.nc
        probe = nc.sync.nop()
        wait_clock.add_sem_waits(
            probe.ins, ScopedClock({None: tick_clock.global_clock})
        )
        si = probe.ins.sync_info
        waits = list(si.on_wait) if si is not None else []
        if si is not None:
            probe.ins.sync_info = mybir.SyncInfo(
                on_wait=[], on_update=list(si.on_update)
            )
        for w in waits:
            wi = nc.sync.nop()
            wi.ins.sync_info = mybir.SyncInfo(on_wait=[w], on_update=[])
        nc.sync.drain()

        nc.all_engine_barrier()
        assert self.sems is not None
        popped = nc._tile_sem_poison_stack.pop()
        assert popped is self._sem_poison
        nc.clear_and_free_semaphores(list(self.sems.allocated().values()))
        nc.all_engine_barrier()

    tile_mod.TileContext._drain_and_barrier = _drain_and_barrier


def _split_waits(nc, max_waits=1):
    """This walrus build allows only one semaphore wait per instruction;
    move excess waits onto nops inserted just before, same engine.  Keep a
    PE-updated semaphore (typically last to arrive) on the instruction
    itself so the chained-nop latency hides behind it."""
    import concourse.mybir as mybir

    ctr = 0
    for f in nc.m.functions:
        for bb in f.blocks:
            insts = bb.instructions
            if not any(
                i.sync_info is not None and len(i.sync_info.on_wait) > max_waits
                for i in insts
            ):
                continue
            new = []
            for inst in insts:
                si = inst.sync_info
                ws = list(si.on_wait) if si is not None else []
                if len(ws) > max_waits:
                    ws.sort(key=lambda w: "PE" in (w.ant_name or ""))
                    extra, keep = ws[:-max_waits], ws[-max_waits:]
                    for j in range(0, len(extra), max_waits):
                        ctr += 1
                        nop = mybir.InstNoOp(
                            name=f"waitsplit-{ctr}",
                            sync_info=mybir.SyncInfo(
                                on_wait=extra[j:j + max_waits], on_update=[]
                            ),
                            bass_nofuse=True,
                            engine=inst.engine,
                        )
                        new.append(nop)
                    inst.sync_info = mybir.SyncInfo(
                        on_wait=keep, on_update=list(si.on_update)
                    )
                new.append(inst)
            bb.instructions = new


# ---------------------------------------------------------------------------
# program construction
# ---------------------------------------------------------------------------

def _build_program():
    import concourse.bass as bass
    import concourse.mybir as mybir
    from concourse.alu_op_type import AluOpType
    from concourse.tile import TileContext

    _patch_tile_drain()

    f32 = mybir.dt.float32
    bf = mybir.dt.float16
    u32 = mybir.dt.uint32

    nc = bass.Bass()
    x_in = nc.declare_dram_parameter("x", [C, H * W], bf, isOutput=False)
    w_in = {}
    for nm in ("wd", "wu", "wr", "wl"):
        w_in[nm] = nc.declare_dram_parameter(nm, [C, K * C], bf, isOutput=False)
    # w-major output: y[c, w*H + h] fp16; host transposes + upcasts
    y_out = nc.declare_dram_parameter("y", [C, W * H], bf, isOutput=True)

    IMGW = IB + H * RS + 4

    with TileContext(nc) as tc:
        with (
            tc.tile_pool(name="img", bufs=1) as imgp,
            tc.tile_pool(name="wpool", bufs=1) as wp,
            tc.tile_pool(name="cpool", bufs=1) as cp,
            tc.tile_pool(name="stage", bufs=1) as sp,
            tc.tile_pool(name="psum", bufs=1, space="PSUM") as pp,
            tc.tile_pool(name="fpsum", bufs=1, space="PSUM") as fp,
        ):
            # wd first: the first scan stalls on it; the other weights and
            # x stream behind it
            wt = {}
            for nm in ("wd", "wu", "wr", "wl"):
                wt[nm] = wp.tile([C, K * C], bf, tag=f"wt_{nm}", name=f"wt_{nm}")
            nc.sync.dma_start(out=wt["wd"][:], in_=w_in["wd"][:])

            img = imgp.tile([C, IMGW], bf, tag="img")
            img3 = img[:, IB:IB + H * RS].rearrange("p (h r) -> p h r", r=RS)
            # zero pads + per-row guard columns
            nc.vector.memset(img[:, 0:IB].bitcast(u32), 0)
            nc.vector.memset(img[:, IB + H * RS:].bitcast(u32), 0)
            nc.vector.memset(img3[:, :, 0:1], 0)
            nc.vector.memset(img3[:, :, 257:260], 0)
            # load x into the data region, 16-row blocks
            x3 = x_in.rearrange("p (h w) -> p h w", w=W)
            # rows 0-3 first so the scan can start while the rest streams
            nc.sync.dma_start(out=img3[:, 0:4, 1:257], in_=x3[:, 0:4, :])
            nc.sync.dma_start(out=img3[:, 4:16, 1:257], in_=x3[:, 4:16, :])
            for nm in ("wu", "wr", "wl"):
                nc.sync.dma_start(out=wt[nm][:], in_=w_in[nm][:])
            for hb in range(16, H, 16):
                nc.sync.dma_start(
                    out=img3[:, hb:hb + 16, 1:257], in_=x3[:, hb:hb + 16, :]
                )

            # carry slots for the right scan
            cts = []
            for ci in range(3):
                t = cp.tile([C, CT], bf, tag=f"ct{ci}", name=f"ct{ci}")
                nc.vector.memset(t[:].bitcast(u32), 0)
                cts.append(t)
            # w-major guarded staging slots for the left scan
            stg = []
            for ci in range(2):
                t = sp.tile([C, SBLK * CT], bf, tag=f"stg{ci}", name=f"stg{ci}")
                t3 = t.rearrange("p (s r) -> p s r", r=CT)
                nc.vector.memset(t3[:, :, 0:4].bitcast(u32), 0)
                nc.vector.memset(t3[:, :, 132:136].bitcast(u32), 0)
                stg.append(t)

            # psum tiles: one per chunk, single-buffered (WAR == RAW dep)
            def mk_ps(tag, m):
                ts = []
                for k in range(len(m) - 1):
                    wk = (m[k + 1] + 4 if k < len(m) - 2 else m[-1]) - m[k]
                    ts.append(pp.tile([C, wk], f32, tag=f"{tag}{k}",
                                      name=f"{tag}{k}"))
                return ts

            pd = mk_ps("pd", M_DU)
            pr = mk_ps("pr", M_RL)
            fps = fp.tile([C, 256], f32, tag="fps", name="fps")

            filler_rhs = wt["wd"][:, 0:256]

            def fillers(n, fw):
                for fi in range(n):
                    nc.tensor.matmul(
                        fps[:, 0:fw], wt["wd"][:, fi * C:(fi + 1) * C],
                        filler_rhs[:, 0:fw], start=True, stop=True,
                    )

            def row_base(h):
                # img col index of row h's data col 0
                return IB + h * RS + 1

            def chunk_taps(wtile, m, ps_tiles, rhs_base_fn):
                """Emit per-chunk tap matmuls. rhs_base_fn(off) -> AP for
                [C, width] rhs starting at data col `off` (may be negative:
                guards)."""
                nch = len(m) - 1
                for k in range(nch):
                    a = m[k]
                    bw = (m[k + 1] + 4 if k < nch - 1 else m[-1]) - a
                    for t in range(K):
                        s = t - 4
                        nc.tensor.matmul(
                            ps_tiles[k][:, 0:bw],
                            wtile[:, t * C:(t + 1) * C],
                            rhs_base_fn(a + s, bw),
                            start=(t == 0), stop=(t == K - 1),
                        )

            def chunk_drains(m, ps_tiles, out_fn, x_fn):
                """Per-chunk fused drains: out = max(psum,0) + x.
                out_fn/x_fn(lo, hi) -> AP covering out cols [lo, hi)."""
                nch = len(m) - 1
                for k in range(nch):
                    lo = m[k] + (4 if k > 0 else 0)
                    hi = m[k + 1] + (4 if k < nch - 1 else 0)
                    plo = lo - m[k]
                    nc.vector.scalar_tensor_tensor(
                        out=out_fn(lo, hi),
                        in0=ps_tiles[k][:, plo:plo + hi - lo],
                        scalar=0.0,
                        in1=x_fn(lo, hi),
                        op0=AluOpType.max, op1=AluOpType.add,
                    )

            # ---------------- phase 1 down / phase 2 up --------------------
            for phase, wname, order in (
                (1, "wd", range(1, H)),
                (2, "wu", range(H - 2, -1, -1)),
            ):
                src_off = -1 if phase == 1 else 1
                for i in order:
                    sb = row_base(i + src_off)
                    db = row_base(i)

                    chunk_taps(
                        wt[wname], M_DU, pd,
                        lambda off, bw: img[:, sb + off: sb + off + bw],
                    )
                    chunk_drains(
                        M_DU, pd,
                        lambda lo, hi: img[:, db + lo: db + hi],
                        lambda lo, hi: img[:, db + lo: db + hi],
                    )
                    fillers(NF_DU, FW_DU)

            # ---------------- phase 3: right -------------------------------
            def img_col(w, lo, hi):
                # [C, hi-lo] strided view of image column w, rows [lo, hi)
                return img3[:, lo:hi, 1 + w]

            nc.scalar.copy(cts[0][:, 4:132], img_col(0, 0, H))
            for w in range(1, W):
                prev, new = cts[(w - 1) % 3], cts[w % 3]
                chunk_taps(
                    wt["wr"], M_RL, pr,
                    lambda off, bw: prev[:, 4 + off: 4 + off + bw],
                )
                chunk_drains(
                    M_RL, pr,
                    lambda lo, hi: new[:, 4 + lo: 4 + hi],
                    lambda lo, hi: img_col(w, lo, hi),
                )
                # persist for phase 4's +x reads
                nc.scalar.copy(img_col(w, 0, H), new[:, 4:132])
                fillers(NF_RL, FW_RL)

            # ---------------- phase 4: left (stores overlap) ---------------
            def slot(w):
                b = w // SBLK
                return stg[b % 2][:, (w - b * SBLK) * CT:(w - b * SBLK + 1) * CT]

            def flush(b):
                t3 = stg[b % 2].rearrange("p (s r) -> p s r", r=CT)
                nc.sync.dma_start(
                    out=y_out[:, b * SBLK * H:(b + 1) * SBLK * H],
                    in_=t3[:, :, 4:132],
                )

            nc.scalar.copy(slot(W - 1)[:, 4:132], img_col(W - 1, 0, H))
            for w in range(W - 2, -1, -1):
                prev, new = slot(w + 1), slot(w)
                chunk_taps(
                    wt["wl"], M_RL, pr,
                    lambda off, bw: prev[:, 4 + off: 4 + off + bw],
                )
                chunk_drains(
                    M_RL, pr,
                    lambda lo, hi: new[:, 4 + lo: 4 + hi],
                    lambda lo, hi: img_col(w, lo, hi),
                )
                if w % SBLK == 0:
                    flush(w // SBLK)
                fillers(NF_RL, FW_RL)

    _split_waits(nc, max_waits=1)
    return nc


def _get_program():
    key = "prog"
    if key not in _CACHE:
        _CACHE[key] = _build_program()
    return _CACHE[key]


# ---------------------------------------------------------------------------
# entry point
# ---------------------------------------------------------------------------

def _prep_w(w):
    # w: (Cout, Cin, K) -> lhsT layout [Cin, K*Cout], fp16
    return np.ascontiguousarray(
        np.transpose(np.asarray(w, np.float32), (1, 2, 0)).reshape(C, K * C)
    ).astype(np.float16)


def _make_in_map(x_img, ws):
    # x_img: (C, H, W); ws: [w_down, w_up, w_right, w_left]
    wd, wu, wr, wl = (_prep_w(w) for w in ws)
    return {
        "x": np.ascontiguousarray(
            np.asarray(x_img, np.float32).reshape(C, H * W)
        ).astype(np.float16),
        "wd": wd, "wu": wu, "wr": wr, "wl": wl,
    }


def _postprocess(y_flat):
    # y is w-major fp16 [C, W*H]; transpose back to [C, H, W] fp32
    return (
        np.asarray(y_flat).astype(np.float32).reshape(C, W, H).transpose(0, 2, 1)
    )


def kernel(x, w_down, w_up, w_right, w_left, _trace=False):
    from concourse.bass_utils import run_bass_kernel_spmd

    nc = _get_program()

    x = np.asarray(x, np.float32)
    ws = [w_down, w_up, w_right, w_left]
    in_maps = [_make_in_map(x[b], ws) for b in range(B)]
    res = run_bass_kernel_spmd(nc, in_maps, list(range(N_CORES)), trace=_trace)
    out = np.stack([_postprocess(res.results[b]["y"]) for b in range(B)])
    if _trace:
        return out, res
    return out


# revision 9
# speedup vs baseline: 1.0665x; 1.0007x over previous
"""Message-passing kernel for Trainium2 (8 NeuronCores, data-parallel over batch).

Reference computation (per batch element, C=128 channels, H=128, W=256):
  4 sequential directional scans (down, up, right, left); each scan step is
    out[i] = x[i] + relu(conv1d(out[i-1]))
  with a 'same'-padded K=9 conv1d (C->C) along the non-scan spatial axis.

Design (per core, one batch element), v2:
  - everything fp16 except PSUM (fp32) and drain arithmetic: 16-bit matmuls
    run at full stream rate at ANY width (fp32r needs >=256), enabling
    chunked, software-pipelined steps; fp16 over bf16 for the extra
    mantissa bits (measured rel err ~7e-4 vs 6e-3).
  - image resident in SBUF as [C=128 partitions, 4 + H*260 + 4] fp16
    (per-row: 1 zero guard, 256 data, 3 zero guards; plus 4-col pads at
    both ends) so every row sees 4 zeros on each side.
  - each scan step is split into chunks along the output row; each chunk
    has its own psum tile and 9 fixed-psum / sliding-rhs tap matmuls
    (start/stop per chunk group); chunks overlap by 4 cols (redundant
    compute) so chunk k's drain feeds chunk k's AND k+1's next-step taps.
  - drain = fused max(psum,0)+x on DVE (scalar_tensor_tensor); the next
    step's chunk-k taps wait only on drain-k of the previous step ->
    the DVE+sem+PE-latency window hides behind the other chunks' matmuls.
  - right/left scans: contiguous guarded carry slots [C,136] (pass 3,
    rotating x3) or guarded staging slots (pass 4, [C, SBLK*136] x2,
    DMA-flushed per 32-column block, w-major fp16 out; host transposes
    and upcasts). +x read directly from the image via strided in1.
  - filler matmuls into a scratch psum keep the PE p-state warm through
    each step's drain window.
"""

import numpy as np

C = 128
H = 128
W = 256
K = 9
RS = 260          # image row stride (fp16 words)
IB = 4            # image global base pad
CT = 136          # carry slot width: 4 zeros, 128 data, 4 zeros
B = 8
N_CORES = 8
SBLK = 8          # output staging block (columns)

# chunk boundaries (even) for down/up (row width 256) and right/left (128),
# tuned on HW (probe3-5): du 3 chunks = 1131 ns/step, rl 2 chunks = 964.
M_DU = (0, 88, 176, 256)
M_RL = (0, 64, 128)
NF_DU = 0         # fillers per down/up step (PE-bound; none needed)
FW_DU = 128
NF_RL = 1         # fillers per right/left step (window-bound; p-state insurance)
FW_RL = 128

_CACHE = {}


# ---------------------------------------------------------------------------
# workarounds for this walrus build (exit drain / per-instruction wait limits)
# ---------------------------------------------------------------------------

def _patch_tile_drain():
    import concourse.mybir as mybir
    import concourse.tile as tile_mod
    from concourse.vector_clock import ScopedClock

    def _drain_and_barrier(self, tick_clock, wait_clock):
        nc = self.nc
        probe = nc.sync.nop()
        wait_clock.add_sem_waits(
            probe.ins, ScopedClock({None: tick_clock.global_clock})
        )
        si = probe.ins.sync_info
        waits = list(si.on_wait) if si is not None else []
        if si is not None:
            probe.ins.sync_info = mybir.SyncInfo(
                on_wait=[], on_update=list(si.on_update)
            )
        for w in waits:
            wi = nc.sync.nop()
            wi.ins.sync_info = mybir.SyncInfo(on_wait=[w], on_update=[])
        nc.sync.drain()

        nc.all_engine_barrier()
        assert self.sems is not None
        popped = nc._tile_sem_poison_stack.pop()
        assert popped is self._sem_poison
        nc.clear_and_free_semaphores(list(self.sems.allocated().values()))
        nc.all_engine_barrier()

    tile_mod.TileContext._drain_and_barrier = _drain_and_barrier


def _split_waits(nc, max_waits=1):
    """This walrus build allows only one semaphore wait per instruction;
    move excess waits onto nops inserted just before, same engine.  Keep a
    PE-updated semaphore (typically last to arrive) on the instruction
    itself so the chained-nop latency hides behind it."""
    import concourse.mybir as mybir

    ctr = 0
    for f in nc.m.functions:
        for bb in f.blocks:
            insts = bb.instructions
            if not any(
                i.sync_info is not None and len(i.sync_info.on_wait) > max_waits
                for i in insts
            ):
                continue
            new = []
            for inst in insts:
                si = inst.sync_info
                ws = list(si.on_wait) if si is not None else []
                if len(ws) > max_waits:
                    ws.sort(key=lambda w: "PE" in (w.ant_name or ""))
                    extra, keep = ws[:-max_waits], ws[-max_waits:]
                    for j in range(0, len(extra), max_waits):
                        ctr += 1
                        nop = mybir.InstNoOp(
                            name=f"waitsplit-{ctr}",
                            sync_info=mybir.SyncInfo(
                                on_wait=extra[j:j + max_waits], on_update=[]
                            ),
                            bass_nofuse=True,
                            engine=inst.engine,
                        )
                        new.append(nop)
                    inst.sync_info = mybir.SyncInfo(
                        on_wait=keep, on_update=list(si.on_update)
                    )
                new.append(inst)
            bb.instructions = new


# ---------------------------------------------------------------------------
# program construction
# ---------------------------------------------------------------------------

def _build_program():
    import concourse.bass as bass
    import concourse.mybir as mybir
    from concourse.alu_op_type import AluOpType
    from concourse.tile import TileContext

    _patch_tile_drain()

    f32 = mybir.dt.float32
    bf = mybir.dt.float16
    u32 = mybir.dt.uint32

    nc = bass.Bass()
    x_in = nc.declare_dram_parameter("x", [C, H * W], bf, isOutput=False)
    w_in = {}
    for nm in ("wd", "wu", "wr", "wl"):
        w_in[nm] = nc.declare_dram_parameter(nm, [C, K * C], bf, isOutput=False)
    # w-major output: y[c, w*H + h] fp16; host transposes + upcasts
    y_out = nc.declare_dram_parameter("y", [C, W * H], bf, isOutput=True)

    IMGW = IB + H * RS + 4

    with TileContext(nc) as tc:
        with (
            tc.tile_pool(name="img", bufs=1) as imgp,
            tc.tile_pool(name="wpool", bufs=1) as wp,
            tc.tile_pool(name="cpool", bufs=1) as cp,
            tc.tile_pool(name="stage", bufs=1) as sp,
            tc.tile_pool(name="psum", bufs=1, space="PSUM") as pp,
            tc.tile_pool(name="fpsum", bufs=1, space="PSUM") as fp,
        ):
            # wd first: the first scan stalls on it; the other weights and
            # x stream behind it
            wt = {}
            for nm in ("wd", "wu", "wr", "wl"):
                wt[nm] = wp.tile([C, K * C], bf, tag=f"wt_{nm}", name=f"wt_{nm}")
            nc.sync.dma_start(out=wt["wd"][:], in_=w_in["wd"][:])

            img = imgp.tile([C, IMGW], bf, tag="img")
            img3 = img[:, IB:IB + H * RS].rearrange("p (h r) -> p h r", r=RS)
            # zero pads + per-row guard columns
            nc.vector.memset(img[:, 0:IB].bitcast(u32), 0)
            nc.vector.memset(img[:, IB + H * RS:].bitcast(u32), 0)
            nc.vector.memset(img3[:, :, 0:1], 0)
            nc.vector.memset(img3[:, :, 257:260], 0)
            # load x into the data region, 16-row blocks
            x3 = x_in.rearrange("p (h w) -> p h w", w=W)
            # rows 0-3 first so the scan can start while the rest streams
            nc.sync.dma_start(out=img3[:, 0:4, 1:257], in_=x3[:, 0:4, :])
            nc.sync.dma_start(out=img3[:, 4:12, 1:257], in_=x3[:, 4:12, :])
            for nm in ("wu", "wr", "wl"):
                nc.sync.dma_start(out=wt[nm][:], in_=w_in[nm][:])
            for hb in range(12, H, 8):
                he = min(hb + 8, H)
                nc.sync.dma_start(
                    out=img3[:, hb:he, 1:257], in_=x3[:, hb:he, :]
                )

            # carry slots for the right scan
            cts = []
            for ci in range(3):
                t = cp.tile([C, CT], bf, tag=f"ct{ci}", name=f"ct{ci}")
                nc.vector.memset(t[:].bitcast(u32), 0)
                cts.append(t)
            # w-major guarded staging slots for the left scan
            stg = []
            for ci in range(2):
                t = sp.tile([C, SBLK * CT], bf, tag=f"stg{ci}", name=f"stg{ci}")
                t3 = t.rearrange("p (s r) -> p s r", r=CT)
                nc.vector.memset(t3[:, :, 0:4].bitcast(u32), 0)
                nc.vector.memset(t3[:, :, 132:136].bitcast(u32), 0)
                stg.append(t)

            # psum tiles: one per chunk, single-buffered (WAR == RAW dep)
            def mk_ps(tag, m):
                ts = []
                for k in range(len(m) - 1):
                    wk = (m[k + 1] + 4 if k < len(m) - 2 else m[-1]) - m[k]
                    ts.append(pp.tile([C, wk], f32, tag=f"{tag}{k}",
                                      name=f"{tag}{k}"))
                return ts

            pd = mk_ps("pd", M_DU)
            pr = mk_ps("pr", M_RL)
            fps = fp.tile([C, 256], f32, tag="fps", name="fps")

            filler_rhs = wt["wd"][:, 0:256]

            def fillers(n, fw):
                for fi in range(n):
                    nc.tensor.matmul(
                        fps[:, 0:fw], wt["wd"][:, fi * C:(fi + 1) * C],
                        filler_rhs[:, 0:fw], start=True, stop=True,
                    )

            def row_base(h):
                # img col index of row h's data col 0
                return IB + h * RS + 1

            def chunk_taps(wtile, m, ps_tiles, rhs_base_fn):
                """Emit per-chunk tap matmuls. rhs_base_fn(off) -> AP for
                [C, width] rhs starting at data col `off` (may be negative:
                guards)."""
                nch = len(m) - 1
                for k in range(nch):
                    a = m[k]
                    bw = (m[k + 1] + 4 if k < nch - 1 else m[-1]) - a
                    for t in range(K):
                        s = t - 4
                        nc.tensor.matmul(
                            ps_tiles[k][:, 0:bw],
                            wtile[:, t * C:(t + 1) * C],
                            rhs_base_fn(a + s, bw),
                            start=(t == 0), stop=(t == K - 1),
                        )

            def chunk_drains(m, ps_tiles, out_fn, x_fn):
                """Per-chunk fused drains: out = max(psum,0) + x.
                out_fn/x_fn(lo, hi) -> AP covering out cols [lo, hi)."""
                nch = len(m) - 1
                for k in range(nch):
                    lo = m[k] + (4 if k > 0 else 0)
                    hi = m[k + 1] + (4 if k < nch - 1 else 0)
                    plo = lo - m[k]
                    nc.vector.scalar_tensor_tensor(
                        out=out_fn(lo, hi),
                        in0=ps_tiles[k][:, plo:plo + hi - lo],
                        scalar=0.0,
                        in1=x_fn(lo, hi),
                        op0=AluOpType.max, op1=AluOpType.add,
                    )

            # ---------------- phase 1 down / phase 2 up --------------------
            for phase, wname, order in (
                (1, "wd", range(1, H)),
                (2, "wu", range(H - 2, -1, -1)),
            ):
                src_off = -1 if phase == 1 else 1
                for i in order:
                    sb = row_base(i + src_off)
                    db = row_base(i)

                    chunk_taps(
                        wt[wname], M_DU, pd,
                        lambda off, bw: img[:, sb + off: sb + off + bw],
                    )
                    chunk_drains(
                        M_DU, pd,
                        lambda lo, hi: img[:, db + lo: db + hi],
                        lambda lo, hi: img[:, db + lo: db + hi],
                    )
                    fillers(NF_DU, FW_DU)

            # ---------------- phase 3: right -------------------------------
            def img_col(w, lo, hi):
                # [C, hi-lo] strided view of image column w, rows [lo, hi)
                return img3[:, lo:hi, 1 + w]

            nc.scalar.copy(cts[0][:, 4:132], img_col(0, 0, H))
            for w in range(1, W):
                prev, new = cts[(w - 1) % 3], cts[w % 3]
                chunk_taps(
                    wt["wr"], M_RL, pr,
                    lambda off, bw: prev[:, 4 + off: 4 + off + bw],
                )
                chunk_drains(
                    M_RL, pr,
                    lambda lo, hi: new[:, 4 + lo: 4 + hi],
                    lambda lo, hi: img_col(w, lo, hi),
                )
                # persist for phase 4's +x reads
                nc.scalar.copy(img_col(w, 0, H), new[:, 4:132])
                fillers(NF_RL, FW_RL)

            # ---------------- phase 4: left (stores overlap) ---------------
            def slot(w):
                b = w // SBLK
                return stg[b % 2][:, (w - b * SBLK) * CT:(w - b * SBLK + 1) * CT]

            def flush(b):
                t3 = stg[b % 2].rearrange("p (s r) -> p s r", r=CT)
                nc.sync.dma_start(
                    out=y_out[:, b * SBLK * H:(b + 1) * SBLK * H],
                    in_=t3[:, :, 4:132],
                )

            nc.scalar.copy(slot(W - 1)[:, 4:132], img_col(W - 1, 0, H))
            for w in range(W - 2, -1, -1):
                prev, new = slot(w + 1), slot(w)
                chunk_taps(
                    wt["wl"], M_RL, pr,
                    lambda off, bw: prev[:, 4 + off: 4 + off + bw],
                )
                chunk_drains(
                    M_RL, pr,
                    lambda lo, hi: new[:, 4 + lo: 4 + hi],
                    lambda lo, hi: img_col(w, lo, hi),
                )
                if w % SBLK == 0:
                    flush(w // SBLK)
                fillers(NF_RL, FW_RL)

    _split_waits(nc, max_waits=1)
    return nc


def _get_program():
    key = "prog"
    if key not in _CACHE:
        _CACHE[key] = _build_program()
    return _CACHE[key]


# ---------------------------------------------------------------------------
# entry point
# ---------------------------------------------------------------------------

def _prep_w(w):
    # w: (Cout, Cin, K) -> lhsT layout [Cin, K*Cout], fp16
    return np.ascontiguousarray(
        np.transpose(np.asarray(w, np.float32), (1, 2, 0)).reshape(C, K * C)
    ).astype(np.float16)


def _make_in_map(x_img, ws):
    # x_img: (C, H, W); ws: [w_down, w_up, w_right, w_left]
    wd, wu, wr, wl = (_prep_w(w) for w in ws)
    return {
        "x": np.ascontiguousarray(
            np.asarray(x_img, np.float32).reshape(C, H * W)
        ).astype(np.float16),
        "wd": wd, "wu": wu, "wr": wr, "wl": wl,
    }


def _postprocess(y_flat):
    # y is w-major fp16 [C, W*H]; transpose back to [C, H, W] fp32
    return (
        np.asarray(y_flat).astype(np.float32).reshape(C, W, H).transpose(0, 2, 1)
    )


def kernel(x, w_down, w_up, w_right, w_left, _trace=False):
    from concourse.bass_utils import run_bass_kernel_spmd

    nc = _get_program()

    x = np.asarray(x, np.float32)
    ws = [w_down, w_up, w_right, w_left]
    in_maps = [_make_in_map(x[b], ws) for b in range(B)]
    res = run_bass_kernel_spmd(nc, in_maps, list(range(N_CORES)), trace=_trace)
    out = np.stack([_postprocess(res.results[b]["y"]) for b in range(B)])
    if _trace:
        return out, res
    return out


# revision 10
# speedup vs baseline: 1.0689x; 1.0022x over previous
"""Message-passing kernel for Trainium2 (8 NeuronCores, data-parallel over batch).

Reference computation (per batch element, C=128 channels, H=128, W=256):
  4 sequential directional scans (down, up, right, left); each scan step is
    out[i] = x[i] + relu(conv1d(out[i-1]))
  with a 'same'-padded K=9 conv1d (C->C) along the non-scan spatial axis.

Design (per core, one batch element), v2:
  - everything fp16 except PSUM (fp32) and drain arithmetic: 16-bit matmuls
    run at full stream rate at ANY width (fp32r needs >=256), enabling
    chunked, software-pipelined steps; fp16 over bf16 for the extra
    mantissa bits (measured rel err ~7e-4 vs 6e-3).
  - image resident in SBUF as [C=128 partitions, 4 + H*260 + 4] fp16
    (per-row: 1 zero guard, 256 data, 3 zero guards; plus 4-col pads at
    both ends) so every row sees 4 zeros on each side.
  - each scan step is split into chunks along the output row; each chunk
    has its own psum tile and 9 fixed-psum / sliding-rhs tap matmuls
    (start/stop per chunk group); chunks overlap by 4 cols (redundant
    compute) so chunk k's drain feeds chunk k's AND k+1's next-step taps.
  - drain = fused max(psum,0)+x on DVE (scalar_tensor_tensor); the next
    step's chunk-k taps wait only on drain-k of the previous step ->
    the DVE+sem+PE-latency window hides behind the other chunks' matmuls.
  - right/left scans: contiguous guarded carry slots [C,136] (pass 3,
    rotating x3) or guarded staging slots (pass 4, [C, SBLK*136] x2,
    DMA-flushed per 32-column block, w-major fp16 out; host transposes
    and upcasts). +x read directly from the image via strided in1.
  - filler matmuls into a scratch psum keep the PE p-state warm through
    each step's drain window.
"""

import numpy as np

C = 128
H = 128
W = 256
K = 9
RS = 260          # image row stride (fp16 words)
IB = 4            # image global base pad
CT = 136          # carry slot width: 4 zeros, 128 data, 4 zeros
B = 8
N_CORES = 8
SBLK = 8          # output staging block (columns)

# chunk boundaries (even) for down/up (row width 256) and right/left (128),
# tuned on HW (probe3-5): du 3 chunks = 1131 ns/step, rl 2 chunks = 964.
M_DU = (0, 88, 176, 256)
M_RL = (0, 64, 128)
NF_DU = 0         # fillers per down/up step (PE-bound; none needed)
FW_DU = 128
NF_RL = 1         # fillers per right/left step (window-bound; p-state insurance)
FW_RL = 128

_CACHE = {}


# ---------------------------------------------------------------------------
# workarounds for this walrus build (exit drain / per-instruction wait limits)
# ---------------------------------------------------------------------------

def _patch_tile_drain():
    import concourse.mybir as mybir
    import concourse.tile as tile_mod
    from concourse.vector_clock import ScopedClock

    def _drain_and_barrier(self, tick_clock, wait_clock):
        nc = self.nc
        probe = nc.sync.nop()
        wait_clock.add_sem_waits(
            probe.ins, ScopedClock({None: tick_clock.global_clock})
        )
        si = probe.ins.sync_info
        waits = list(si.on_wait) if si is not None else []
        if si is not None:
            probe.ins.sync_info = mybir.SyncInfo(
                on_wait=[], on_update=list(si.on_update)
            )
        for w in waits:
            wi = nc.sync.nop()
            wi.ins.sync_info = mybir.SyncInfo(on_wait=[w], on_update=[])
        nc.sync.drain()

        nc.all_engine_barrier()
        assert self.sems is not None
        popped = nc._tile_sem_poison_stack.pop()
        assert popped is self._sem_poison
        nc.clear_and_free_semaphores(list(self.sems.allocated().values()))
        nc.all_engine_barrier()

    tile_mod.TileContext._drain_and_barrier = _drain_and_barrier


def _split_waits(nc, max_waits=1):
    """This walrus build allows only one semaphore wait per instruction;
    move excess waits onto nops inserted just before, same engine.  Keep a
    PE-updated semaphore (typically last to arrive) on the instruction
    itself so the chained-nop latency hides behind it."""
    import concourse.mybir as mybir

    ctr = 0
    for f in nc.m.functions:
        for bb in f.blocks:
            insts = bb.instructions
            if not any(
                i.sync_info is not None and len(i.sync_info.on_wait) > max_waits
                for i in insts
            ):
                continue
            new = []
            for inst in insts:
                si = inst.sync_info
                ws = list(si.on_wait) if si is not None else []
                if len(ws) > max_waits:
                    ws.sort(key=lambda w: "PE" in (w.ant_name or ""))
                    extra, keep = ws[:-max_waits], ws[-max_waits:]
                    for j in range(0, len(extra), max_waits):
                        ctr += 1
                        nop = mybir.InstNoOp(
                            name=f"waitsplit-{ctr}",
                            sync_info=mybir.SyncInfo(
                                on_wait=extra[j:j + max_waits], on_update=[]
                            ),
                            bass_nofuse=True,
                            engine=inst.engine,
                        )
                        new.append(nop)
                    inst.sync_info = mybir.SyncInfo(
                        on_wait=keep, on_update=list(si.on_update)
                    )
                new.append(inst)
            bb.instructions = new


# ---------------------------------------------------------------------------
# program construction
# ---------------------------------------------------------------------------

def _build_program():
    import concourse.bass as bass
    import concourse.mybir as mybir
    from concourse.alu_op_type import AluOpType
    from concourse.tile import TileContext

    _patch_tile_drain()

    f32 = mybir.dt.float32
    bf = mybir.dt.float16
    u32 = mybir.dt.uint32

    nc = bass.Bass()
    x_in = nc.declare_dram_parameter("x", [C, H * W], bf, isOutput=False)
    w_in = {}
    for nm in ("wd", "wu", "wr", "wl"):
        w_in[nm] = nc.declare_dram_parameter(nm, [C, K * C], bf, isOutput=False)
    # w-major output: y[c, w*H + h] fp16; host transposes + upcasts
    y_out = nc.declare_dram_parameter("y", [C, W * H], bf, isOutput=True)

    IMGW = IB + H * RS + 4

    with TileContext(nc) as tc:
        with (
            tc.tile_pool(name="img", bufs=1) as imgp,
            tc.tile_pool(name="wpool", bufs=1) as wp,
            tc.tile_pool(name="cpool", bufs=1) as cp,
            tc.tile_pool(name="stage", bufs=1) as sp,
            tc.tile_pool(name="psum", bufs=1, space="PSUM") as pp,
            tc.tile_pool(name="fpsum", bufs=1, space="PSUM") as fp,
        ):
            # wd first: the first scan stalls on it; the other weights and
            # x stream behind it
            wt = {}
            for nm in ("wd", "wu", "wr", "wl"):
                wt[nm] = wp.tile([C, K * C], bf, tag=f"wt_{nm}", name=f"wt_{nm}")
            nc.sync.dma_start(out=wt["wd"][:], in_=w_in["wd"][:])

            img = imgp.tile([C, IMGW], bf, tag="img")
            img3 = img[:, IB:IB + H * RS].rearrange("p (h r) -> p h r", r=RS)
            # zero pads + per-row guard columns
            nc.vector.memset(img[:, 0:IB].bitcast(u32), 0)
            nc.vector.memset(img[:, IB + H * RS:].bitcast(u32), 0)
            nc.vector.memset(img3[:, :, 0:1], 0)
            nc.vector.memset(img3[:, :, 257:260], 0)
            # load x into the data region, 16-row blocks
            x3 = x_in.rearrange("p (h w) -> p h w", w=W)
            # rows 0-3 first so the scan can start while the rest streams
            nc.sync.dma_start(out=img3[:, 0:4, 1:257], in_=x3[:, 0:4, :])
            for hb in range(4, H, 8):
                he = min(hb + 8, H)
                nc.sync.dma_start(
                    out=img3[:, hb:he, 1:257], in_=x3[:, hb:he, :]
                )
            # wu/wr/wl aren't needed until pass 2 (~150us); keep them off
            # the x stream's queue
            for nm in ("wu", "wr", "wl"):
                nc.sync.dma_start(out=wt[nm][:], in_=w_in[nm][:])

            # carry slots for the right scan
            cts = []
            for ci in range(3):
                t = cp.tile([C, CT], bf, tag=f"ct{ci}", name=f"ct{ci}")
                nc.vector.memset(t[:].bitcast(u32), 0)
                cts.append(t)
            # w-major guarded staging slots for the left scan
            stg = []
            for ci in range(2):
                t = sp.tile([C, SBLK * CT], bf, tag=f"stg{ci}", name=f"stg{ci}")
                t3 = t.rearrange("p (s r) -> p s r", r=CT)
                nc.vector.memset(t3[:, :, 0:4].bitcast(u32), 0)
                nc.vector.memset(t3[:, :, 132:136].bitcast(u32), 0)
                stg.append(t)

            # psum tiles: one per chunk, single-buffered (WAR == RAW dep)
            def mk_ps(tag, m):
                ts = []
                for k in range(len(m) - 1):
                    wk = (m[k + 1] + 4 if k < len(m) - 2 else m[-1]) - m[k]
                    ts.append(pp.tile([C, wk], f32, tag=f"{tag}{k}",
                                      name=f"{tag}{k}"))
                return ts

            pd = mk_ps("pd", M_DU)
            pr = mk_ps("pr", M_RL)
            fps = fp.tile([C, 256], f32, tag="fps", name="fps")

            filler_rhs = wt["wd"][:, 0:256]

            def fillers(n, fw):
                for fi in range(n):
                    nc.tensor.matmul(
                        fps[:, 0:fw], wt["wd"][:, fi * C:(fi + 1) * C],
                        filler_rhs[:, 0:fw], start=True, stop=True,
                    )

            def row_base(h):
                # img col index of row h's data col 0
                return IB + h * RS + 1

            def chunk_taps(wtile, m, ps_tiles, rhs_base_fn):
                """Emit per-chunk tap matmuls. rhs_base_fn(off) -> AP for
                [C, width] rhs starting at data col `off` (may be negative:
                guards)."""
                nch = len(m) - 1
                for k in range(nch):
                    a = m[k]
                    bw = (m[k + 1] + 4 if k < nch - 1 else m[-1]) - a
                    for t in range(K):
                        s = t - 4
                        nc.tensor.matmul(
                            ps_tiles[k][:, 0:bw],
                            wtile[:, t * C:(t + 1) * C],
                            rhs_base_fn(a + s, bw),
                            start=(t == 0), stop=(t == K - 1),
                        )

            def chunk_drains(m, ps_tiles, out_fn, x_fn):
                """Per-chunk fused drains: out = max(psum,0) + x.
                out_fn/x_fn(lo, hi) -> AP covering out cols [lo, hi)."""
                nch = len(m) - 1
                for k in range(nch):
                    lo = m[k] + (4 if k > 0 else 0)
                    hi = m[k + 1] + (4 if k < nch - 1 else 0)
                    plo = lo - m[k]
                    nc.vector.scalar_tensor_tensor(
                        out=out_fn(lo, hi),
                        in0=ps_tiles[k][:, plo:plo + hi - lo],
                        scalar=0.0,
                        in1=x_fn(lo, hi),
                        op0=AluOpType.max, op1=AluOpType.add,
                    )

            # ---------------- phase 1 down / phase 2 up --------------------
            for phase, wname, order in (
                (1, "wd", range(1, H)),
                (2, "wu", range(H - 2, -1, -1)),
            ):
                src_off = -1 if phase == 1 else 1
                for i in order:
                    sb = row_base(i + src_off)
                    db = row_base(i)

                    chunk_taps(
                        wt[wname], M_DU, pd,
                        lambda off, bw: img[:, sb + off: sb + off + bw],
                    )
                    chunk_drains(
                        M_DU, pd,
                        lambda lo, hi: img[:, db + lo: db + hi],
                        lambda lo, hi: img[:, db + lo: db + hi],
                    )
                    fillers(NF_DU, FW_DU)

            # ---------------- phase 3: right -------------------------------
            def img_col(w, lo, hi):
                # [C, hi-lo] strided view of image column w, rows [lo, hi)
                return img3[:, lo:hi, 1 + w]

            nc.scalar.copy(cts[0][:, 4:132], img_col(0, 0, H))
            for w in range(1, W):
                prev, new = cts[(w - 1) % 3], cts[w % 3]
                chunk_taps(
                    wt["wr"], M_RL, pr,
                    lambda off, bw: prev[:, 4 + off: 4 + off + bw],
                )
                chunk_drains(
                    M_RL, pr,
                    lambda lo, hi: new[:, 4 + lo: 4 + hi],
                    lambda lo, hi: img_col(w, lo, hi),
                )
                # persist for phase 4's +x reads
                nc.scalar.copy(img_col(w, 0, H), new[:, 4:132])
                fillers(NF_RL, FW_RL)

            # ---------------- phase 4: left (stores overlap) ---------------
            def slot(w):
                b = w // SBLK
                return stg[b % 2][:, (w - b * SBLK) * CT:(w - b * SBLK + 1) * CT]

            def flush(b):
                t3 = stg[b % 2].rearrange("p (s r) -> p s r", r=CT)
                nc.sync.dma_start(
                    out=y_out[:, b * SBLK * H:(b + 1) * SBLK * H],
                    in_=t3[:, :, 4:132],
                )

            nc.scalar.copy(slot(W - 1)[:, 4:132], img_col(W - 1, 0, H))
            for w in range(W - 2, -1, -1):
                prev, new = slot(w + 1), slot(w)
                chunk_taps(
                    wt["wl"], M_RL, pr,
                    lambda off, bw: prev[:, 4 + off: 4 + off + bw],
                )
                chunk_drains(
                    M_RL, pr,
                    lambda lo, hi: new[:, 4 + lo: 4 + hi],
                    lambda lo, hi: img_col(w, lo, hi),
                )
                if w % SBLK == 0:
                    flush(w // SBLK)
                fillers(NF_RL, FW_RL)

    _split_waits(nc, max_waits=1)
    return nc


def _get_program():
    key = "prog"
    if key not in _CACHE:
        _CACHE[key] = _build_program()
    return _CACHE[key]


# ---------------------------------------------------------------------------
# entry point
# ---------------------------------------------------------------------------

def _prep_w(w):
    # w: (Cout, Cin, K) -> lhsT layout [Cin, K*Cout], fp16
    return np.ascontiguousarray(
        np.transpose(np.asarray(w, np.float32), (1, 2, 0)).reshape(C, K * C)
    ).astype(np.float16)


def _make_in_map(x_img, ws):
    # x_img: (C, H, W); ws: [w_down, w_up, w_right, w_left]
    wd, wu, wr, wl = (_prep_w(w) for w in ws)
    return {
        "x": np.ascontiguousarray(
            np.asarray(x_img, np.float32).reshape(C, H * W)
        ).astype(np.float16),
        "wd": wd, "wu": wu, "wr": wr, "wl": wl,
    }


def _postprocess(y_flat):
    # y is w-major fp16 [C, W*H]; transpose back to [C, H, W] fp32
    return (
        np.asarray(y_flat).astype(np.float32).reshape(C, W, H).transpose(0, 2, 1)
    )


def kernel(x, w_down, w_up, w_right, w_left, _trace=False):
    from concourse.bass_utils import run_bass_kernel_spmd

    nc = _get_program()

    x = np.asarray(x, np.float32)
    ws = [w_down, w_up, w_right, w_left]
    in_maps = [_make_in_map(x[b], ws) for b in range(B)]
    res = run_bass_kernel_spmd(nc, in_maps, list(range(N_CORES)), trace=_trace)
    out = np.stack([_postprocess(res.results[b]["y"]) for b in range(B)])
    if _trace:
        return out, res
    return out
